# revision 15
# baseline (speedup 1.0000x reference)
"""Chamfer (GCC) loss kernel for Trainium2, 8-core data-parallel.

Problem: gt, pred: (16, 4096, 3) fp32. loss = mean(min_m d2[b,n,m]) + mean(min_n d2[b,n,m])
with d2 the squared pairwise distance matrix per batch.

Strategy per core (2 batches/core, data-parallel over batch):
  - d2 tiles are produced by the PE as a K=24 bf16 matmul. Each fp32 factor is
    split into 3 bf16 levels (hi/mid/lo) on the host; the 6 dominant cross
    products per coordinate plus 3-level xx and yy rows reproduce
    d2 = xx + yy - 2*gt.pred to ~1e-5 abs while streaming at full PE rate
    (bf16 matmul cost is output-bound, not K-bound).
  - ACT (scalar engine) drains PSUM -> SBUF fp16 (the only engine free to do it).
  - DVE row-min: TT-min tree (2x_1p fp16) per [128,2048] chunk + one
    reduce_min, written per n-block into rowminsA/B[128, 32].
  - DVE col-min: running tensor_tensor(min) into colacc[128, 4096] fp16.
  - Col tail: PE transposes colacc 128x128 blocks into PSUM fp16, one
    reduce_min gives colmins[128, 32].
  - clamp max(d2,0) commutes with min -> applied after, via Relu on ACT with
    fused sum accumulation; partition sums via ones-vector matmul.
    Per-core scalar partial -> host sum / (B*N).
"""

import numpy as np

B = 16
N = 4096
NCORES = 8
BL = B // NCORES          # batches per core
NB = N // 128             # 32 n-blocks
CH = 2048                 # m-chunk width (4 psum banks)
NH = N // CH              # 2 chunks per n-block row
MMN = 512                 # matmul moving free dim (1 psum bank)
KR = 24                   # matmul contraction rows

_CACHE = {}


def _build_nc(n_batches=BL, n_nblocks=NB, do_tail=True):
    from contextlib import ExitStack
    import concourse.bacc as bacc
    import concourse.tile as tile
    from concourse import mybir, masks

    f32 = mybir.dt.float32
    bf16 = mybir.dt.bfloat16
    f16 = mybir.dt.float16
    MIN_OP = mybir.AluOpType.min

    nc = bacc.Bacc("TRN2", target_bir_lowering=False, debug=False)
    gtp_d = nc.dram_tensor("gtp", [BL, KR, N], bf16, kind="ExternalInput")
    prd_d = nc.dram_tensor("prd", [BL, KR, N], bf16, kind="ExternalInput")
    out_d = nc.dram_tensor("out", [1, 1], f32, kind="ExternalOutput")

    with tile.TileContext(nc) as tc, ExitStack() as ctx:
        io = ctx.enter_context(tc.tile_pool(name="io", bufs=1))
        chunks = ctx.enter_context(tc.tile_pool(name="chunks", bufs=4))
        tree = ctx.enter_context(tc.tile_pool(name="tree", bufs=3))
        accs = ctx.enter_context(tc.tile_pool(name="accs", bufs=3))
        psum = ctx.enter_context(tc.tile_pool(name="psum", bufs=2, space="PSUM"))
        psumt = ctx.enter_context(tc.tile_pool(name="psumt", bufs=1, space="PSUM"))

        # constants
        ident = io.tile([128, 128], f16, tag="ident")
        masks.make_identity(nc, ident[:])
        ones = io.tile([128, 1], f32, tag="ones")
        nc.gpsimd.memset(ones[:], 1.0)
        scat = io.tile([1, 2 * BL], f32, tag="scat")

        # inputs
        gts = []
        prds = []
        for b in range(BL):
            gt_sb = io.tile([KR, N], bf16, tag=f"gt{b}")
            prd_sb = io.tile([KR, N], bf16, tag=f"prd{b}")
            if b == 0:
                # first-needed slices land first so the PE can start early
                nc.sync.dma_start(gt_sb[:, 0:128], gtp_d[0][:, 0:128])
                nc.scalar.dma_start(prd_sb[:, 0:CH], prd_d[0][:, 0:CH])
                nc.sync.dma_start(prd_sb[:, CH:N], prd_d[0][:, CH:N])
                nc.scalar.dma_start(gt_sb[:, 128:N], gtp_d[0][:, 128:N])
            else:
                nc.gpsimd.dma_start(gt_sb[:], gtp_d[b])
                nc.gpsimd.dma_start(prd_sb[:], prd_d[b])
            gts.append(gt_sb)
            prds.append(prd_sb)

        def emit_tail(b, rowminsA, colacc):
            # dist1: sum over n of relu(rowmins)
            relu1 = accs.tile([128, NB], f32, tag="relu1")
            s1 = accs.tile([128, 1], f32, tag="s1")
            nc.scalar.activation(relu1[:], rowminsA[:],
                                 mybir.ActivationFunctionType.Relu,
                                 accum_out=s1[:])
            ps11 = psumt.tile([1, 1], f32, tag="pst")
            nc.tensor.matmul(ps11[:], ones[:], s1[:], start=True, stop=True)
            nc.vector.tensor_copy(scat[:, 2 * b:2 * b + 1], ps11[:])

            # dist2: transpose colacc, min over partitions, relu-sum.
            # pst has its own 2-bank psum tag so the tail never displaces the
            # main pipeline's psum slots.
            cmins = accs.tile([128, NB], f32, tag="cmins")
            for hh in range(2):
                pst = psumt.tile([128, N // 2], f16, tag="pst")
                for blk in range(NB // 2):
                    g = hh * (NB // 2) + blk
                    nc.tensor.transpose(pst[:, blk * 128:(blk + 1) * 128],
                                        colacc[:, g * 128:(g + 1) * 128],
                                        ident[:])
                nc.vector.tensor_reduce(
                    cmins[:, hh * (NB // 2):(hh + 1) * (NB // 2)],
                    pst[:].rearrange("p (b f) -> p b f", f=128),
                    axis=mybir.AxisListType.X, op=MIN_OP)
            relu2 = accs.tile([128, NB], f32, tag="relu2")
            s2 = accs.tile([128, 1], f32, tag="s2")
            nc.scalar.activation(relu2[:], cmins[:],
                                 mybir.ActivationFunctionType.Relu,
                                 accum_out=s2[:])
            ps12 = psumt.tile([1, 1], f32, tag="pst")
            nc.tensor.matmul(ps12[:], ones[:], s2[:], start=True, stop=True)
            nc.vector.tensor_copy(scat[:, 2 * b + 1:2 * b + 2], ps12[:])

        pending_tail = None
        for b in range(n_batches):
            gt_sb, prd_sb = gts[b], prds[b]
            rowminsA = accs.tile([128, NB], f32, tag="rowminsA")
            colacc = accs.tile([128, N], f16, tag="colacc")
            # n-blocks processed in pairs: one [128, 2*N] fp16 staging tile
            # holds both; row-tree ops run on strided [128, 2, X] views so the
            # per-op DVE overhead is paid once per pair.
            CHUNKS = (1536, 1536, 1024)   # 3+3+2 psum banks, leaves 2 for pst
            for nbp in range(n_nblocks // 2):
                if nbp == 1 and pending_tail is not None:
                    emit_tail(*pending_tail)
                    pending_tail = None
                sb = chunks.tile([128, 2 * N], f16, tag="sb")
                for half in range(2):
                    nb = 2 * nbp + half
                    lhsT = gt_sb[:, nb * 128:(nb + 1) * 128]
                    m0 = 0
                    for ch in CHUNKS:
                        ps = psum.tile([128, ch], f32, tag="ps")
                        for j in range(ch // MMN):
                            nc.tensor.matmul(
                                ps[:, j * MMN:(j + 1) * MMN],
                                lhsT,
                                prd_sb[:, m0 + j * MMN:m0 + (j + 1) * MMN],
                                start=True, stop=True,
                            )
                        nc.scalar.copy(sb[:, half * N + m0:half * N + m0 + ch], ps[:])
                        m0 += ch
                # col-direction: pairwise min of the two fresh blocks, then chain
                t = tree.tile([128, N], f16, tag="tcol")
                nc.vector.tensor_tensor(t[:], sb[:, :N], sb[:, N:], op=MIN_OP)
                if nbp == 0:
                    nc.vector.tensor_copy(colacc[:], t[:])
                else:
                    nc.vector.tensor_tensor(colacc[:], colacc[:], t[:], op=MIN_OP)
                # row-direction: TT-min tree over m for both blocks at once
                sbv = sb[:].rearrange("p (b m) -> p b m", b=2)
                r0 = tree.tile([128, 2, N // 2], f16, tag="r0")
                nc.vector.tensor_tensor(r0[:], sbv[:, :, :N // 2], sbv[:, :, N // 2:], op=MIN_OP)
                r1 = tree.tile([128, 2, N // 4], f16, tag="r1")
                nc.vector.tensor_tensor(r1[:], r0[:, :, :N // 4], r0[:, :, N // 4:], op=MIN_OP)
                r2 = tree.tile([128, 2, N // 8], f16, tag="r2")
                nc.vector.tensor_tensor(r2[:], r1[:, :, :N // 8], r1[:, :, N // 8:], op=MIN_OP)
                r3 = tree.tile([128, 2, N // 16], f16, tag="r3")
                nc.vector.tensor_tensor(r3[:], r2[:, :, :N // 16], r2[:, :, N // 16:], op=MIN_OP)
                nc.vector.tensor_reduce(rowminsA[:, 2 * nbp:2 * nbp + 2], r3[:],
                                        axis=mybir.AxisListType.X, op=MIN_OP)

            if not do_tail:
                nc.vector.tensor_copy(scat[:, 2 * b:2 * b + 1], rowminsA[:1, :1])
                nc.vector.tensor_copy(scat[:, 2 * b + 1:2 * b + 2], colacc[:1, :1])
                continue
            pending_tail = (b, rowminsA, colacc)

        if pending_tail is not None:
            emit_tail(*pending_tail)
            pending_tail = None
        res = io.tile([1, 1], f32, tag="res")
        nc.vector.tensor_reduce(res[:], scat[:],
                                axis=mybir.AxisListType.X,
                                op=mybir.AluOpType.add)
        nc.sync.dma_start(out_d[:], res[:])

    nc.compile()
    return nc


def _split3(x):
    """Split fp32 array into 3 bf16 levels (returned as fp32 arrays)."""
    import ml_dtypes
    h = x.astype(ml_dtypes.bfloat16).astype(np.float32)
    r = x - h
    m = r.astype(ml_dtypes.bfloat16).astype(np.float32)
    l = (r - m).astype(ml_dtypes.bfloat16).astype(np.float32)
    return h, m, l


def _prep_inputs(gt, pred):
    import ml_dtypes
    gt = np.asarray(gt, dtype=np.float32)
    pred = np.asarray(pred, dtype=np.float32)
    xx = np.sum(gt * gt, axis=-1)      # (B, N)
    yy = np.sum(pred * pred, axis=-1)  # (B, N)
    onesN = np.ones((N,), dtype=np.float32)

    in_maps = []
    for c in range(NCORES):
        gtp = np.empty((BL, KR, N), dtype=np.float32)
        prd = np.empty((BL, KR, N), dtype=np.float32)
        for b in range(BL):
            g = 2 * c + b
            k = 0
            for cc in range(3):
                g0, g1, g2 = _split3(-2.0 * gt[g, :, cc])
                p0, p1, p2 = _split3(pred[g, :, cc])
                # 6 dominant cross products of (g0+g1+g2)*(p0+p1+p2)
                for gl, pl in ((g0, p0), (g0, p1), (g1, p0),
                               (g0, p2), (g1, p1), (g2, p0)):
                    gtp[b, k] = gl
                    prd[b, k] = pl
                    k += 1
            x0, x1, x2 = _split3(xx[g])
            for xl in (x0, x1, x2):
                gtp[b, k] = xl
                prd[b, k] = onesN
                k += 1
            y0, y1, y2 = _split3(yy[g])
            for yl in (y0, y1, y2):
                gtp[b, k] = onesN
                prd[b, k] = yl
                k += 1
            assert k == KR
        in_maps.append({"gtp": gtp.astype(ml_dtypes.bfloat16),
                        "prd": prd.astype(ml_dtypes.bfloat16)})
    return in_maps


def kernel(gt, pred, trace=False):
    from concourse.bass_utils import run_bass_kernel_spmd

    if "nc" not in _CACHE:
        _CACHE["nc"] = _build_nc()
    nc = _CACHE["nc"]
    in_maps = _prep_inputs(gt, pred)
    r = run_bass_kernel_spmd(nc, in_maps, list(range(NCORES)), trace=trace)
    total = sum(float(m["out"][0, 0]) for m in r.results)
    loss = np.float32(total / (B * N))
    if trace:
        return loss, r
    return loss


# revision 16
# speedup vs baseline: 1.0244x; 1.0244x over previous
"""Chamfer (GCC) loss kernel for Trainium2, 8-core data-parallel.

Problem: gt, pred: (16, 4096, 3) fp32. loss = mean(min_m d2[b,n,m]) + mean(min_n d2[b,n,m])
with d2 the squared pairwise distance matrix per batch.

Strategy per core (2 batches/core, data-parallel over batch):
  - d2 tiles are produced by the PE as a K=24 bf16 matmul. Each fp32 factor is
    split into 3 bf16 levels (hi/mid/lo) on the host; the 6 dominant cross
    products per coordinate plus 3-level xx and yy rows reproduce
    d2 = xx + yy - 2*gt.pred to ~1e-5 abs while streaming at full PE rate
    (bf16 matmul cost is output-bound, not K-bound).
  - ACT (scalar engine) drains PSUM -> SBUF fp16 (the only engine free to do it).
  - DVE row-min: TT-min tree (2x_1p fp16) per [128,2048] chunk + one
    reduce_min, written per n-block into rowminsA/B[128, 32].
  - DVE col-min: running tensor_tensor(min) into colacc[128, 4096] fp16.
  - Col tail: PE transposes colacc 128x128 blocks into PSUM fp16, one
    reduce_min gives colmins[128, 32].
  - clamp max(d2,0) commutes with min -> applied after, via Relu on ACT with
    fused sum accumulation; partition sums via ones-vector matmul.
    Per-core scalar partial -> host sum / (B*N).
"""

import numpy as np

B = 16
N = 4096
NCORES = 8
BL = B // NCORES          # batches per core
NB = N // 128             # 32 n-blocks
CH = 2048                 # m-chunk width (4 psum banks)
NH = N // CH              # 2 chunks per n-block row
MMN = 512                 # matmul moving free dim (1 psum bank)
KR = 24                   # matmul contraction rows

_CACHE = {}


def _build_nc(n_batches=BL, n_nblocks=NB, do_tail=True):
    from contextlib import ExitStack
    import concourse.bacc as bacc
    import concourse.tile as tile
    from concourse import mybir, masks

    f32 = mybir.dt.float32
    bf16 = mybir.dt.bfloat16
    f16 = mybir.dt.float16
    MIN_OP = mybir.AluOpType.min

    nc = bacc.Bacc("TRN2", target_bir_lowering=False, debug=False)
    gtp_d = nc.dram_tensor("gtp", [BL, KR, N], bf16, kind="ExternalInput")
    prd_d = nc.dram_tensor("prd", [BL, KR, N], bf16, kind="ExternalInput")
    out_d = nc.dram_tensor("out", [1, 1], f32, kind="ExternalOutput")

    with tile.TileContext(nc) as tc, ExitStack() as ctx:
        io = ctx.enter_context(tc.tile_pool(name="io", bufs=1))
        chunks = ctx.enter_context(tc.tile_pool(name="chunks", bufs=4))
        tree = ctx.enter_context(tc.tile_pool(name="tree", bufs=3))
        accs = ctx.enter_context(tc.tile_pool(name="accs", bufs=2))
        psum = ctx.enter_context(tc.tile_pool(name="psum", bufs=2, space="PSUM"))

        # constants
        ident = io.tile([128, 128], f16, tag="ident")
        masks.make_identity(nc, ident[:])
        ones = io.tile([128, 1], f32, tag="ones")
        nc.gpsimd.memset(ones[:], 1.0)
        scat = io.tile([1, 2 * BL], f32, tag="scat")

        # inputs
        gts = []
        prds = []
        for b in range(BL):
            gt_sb = io.tile([KR, N], bf16, tag=f"gt{b}")
            prd_sb = io.tile([KR, N], bf16, tag=f"prd{b}")
            if b == 0:
                # first-needed slices land first so the PE can start early
                nc.sync.dma_start(gt_sb[:, 0:128], gtp_d[0][:, 0:128])
                nc.scalar.dma_start(prd_sb[:, 0:CH], prd_d[0][:, 0:CH])
                nc.sync.dma_start(prd_sb[:, CH:N], prd_d[0][:, CH:N])
                nc.scalar.dma_start(gt_sb[:, 128:N], gtp_d[0][:, 128:N])
            else:
                nc.gpsimd.dma_start(gt_sb[:], gtp_d[b])
                nc.gpsimd.dma_start(prd_sb[:], prd_d[b])
            gts.append(gt_sb)
            prds.append(prd_sb)

        def emit_tail(b, rowminsA, colacc):
            # dist1: sum over n of relu(rowmins)
            relu1 = accs.tile([128, NB], f32, tag="relu1")
            s1 = accs.tile([128, 1], f32, tag="s1")
            nc.scalar.activation(relu1[:], rowminsA[:],
                                 mybir.ActivationFunctionType.Relu,
                                 accum_out=s1[:])
            ps11 = psum.tile([1, 1], f32, tag="ps")
            nc.tensor.matmul(ps11[:], ones[:], s1[:], start=True, stop=True)
            nc.vector.tensor_copy(scat[:, 2 * b:2 * b + 1], ps11[:])

            # dist2: transpose colacc, min over partitions, relu-sum
            pst = psum.tile([128, N], f16, tag="ps")
            for blk in range(NB):
                nc.tensor.transpose(pst[:, blk * 128:(blk + 1) * 128],
                                    colacc[:, blk * 128:(blk + 1) * 128],
                                    ident[:])
            cmins = accs.tile([128, NB], f32, tag="cmins")
            nc.vector.tensor_reduce(
                cmins[:], pst[:].rearrange("p (b f) -> p b f", f=128),
                axis=mybir.AxisListType.X, op=MIN_OP)
            relu2 = accs.tile([128, NB], f32, tag="relu2")
            s2 = accs.tile([128, 1], f32, tag="s2")
            nc.scalar.activation(relu2[:], cmins[:],
                                 mybir.ActivationFunctionType.Relu,
                                 accum_out=s2[:])
            ps12 = psum.tile([1, 1], f32, tag="ps")
            nc.tensor.matmul(ps12[:], ones[:], s2[:], start=True, stop=True)
            nc.vector.tensor_copy(scat[:, 2 * b + 1:2 * b + 2], ps12[:])

        pending_tail = None
        for b in range(n_batches):
            gt_sb, prd_sb = gts[b], prds[b]
            rowminsA = accs.tile([128, NB], f32, tag="rowminsA")
            colacc = accs.tile([128, N], f16, tag="colacc")
            for nb in range(n_nblocks):
                if nb == 3 and pending_tail is not None:
                    emit_tail(*pending_tail)
                    pending_tail = None
                lhsT = gt_sb[:, nb * 128:(nb + 1) * 128]
                sb = chunks.tile([128, N], f16, tag="sb")
                for h in range(NH):
                    ps = psum.tile([128, CH], f32, tag="ps")
                    for j in range(CH // MMN):
                        m0 = h * CH + j * MMN
                        nc.tensor.matmul(
                            ps[:, j * MMN:(j + 1) * MMN],
                            lhsT,
                            prd_sb[:, m0:m0 + MMN],
                            start=True, stop=True,
                        )
                    nc.scalar.copy(sb[:, h * CH:(h + 1) * CH], ps[:])
                # col-direction: running elementwise min over n-blocks
                if nb == 0:
                    nc.vector.tensor_copy(colacc[:], sb[:])
                else:
                    nc.vector.tensor_tensor(colacc[:], colacc[:], sb[:], op=MIN_OP)
                # row-direction: TT-min tree over all 4096 m
                r0 = tree.tile([128, N // 2], f16, tag="r0")
                nc.vector.tensor_tensor(r0[:], sb[:, :N // 2], sb[:, N // 2:], op=MIN_OP)
                r1 = tree.tile([128, N // 4], f16, tag="r1")
                nc.vector.tensor_tensor(r1[:], r0[:, :N // 4], r0[:, N // 4:], op=MIN_OP)
                r2 = tree.tile([128, N // 8], f16, tag="r2")
                nc.vector.tensor_tensor(r2[:], r1[:, :N // 8], r1[:, N // 8:], op=MIN_OP)
                r3 = tree.tile([128, N // 16], f16, tag="r3")
                nc.vector.tensor_tensor(r3[:], r2[:, :N // 16], r2[:, N // 16:], op=MIN_OP)
                nc.vector.tensor_reduce(rowminsA[:, nb:nb + 1], r3[:],
                                        axis=mybir.AxisListType.X, op=MIN_OP)

            if not do_tail:
                nc.vector.tensor_copy(scat[:, 2 * b:2 * b + 1], rowminsA[:1, :1])
                nc.vector.tensor_copy(scat[:, 2 * b + 1:2 * b + 2], colacc[:1, :1])
                continue
            pending_tail = (b, rowminsA, colacc)

        if pending_tail is not None:
            emit_tail(*pending_tail)
            pending_tail = None
        res = io.tile([1, 1], f32, tag="res")
        nc.vector.tensor_reduce(res[:], scat[:],
                                axis=mybir.AxisListType.X,
                                op=mybir.AluOpType.add)
        nc.sync.dma_start(out_d[:], res[:])

    nc.compile()
    return nc


def _split3(x):
    """Split fp32 array into 3 bf16 levels (returned as fp32 arrays)."""
    import ml_dtypes
    h = x.astype(ml_dtypes.bfloat16).astype(np.float32)
    r = x - h
    m = r.astype(ml_dtypes.bfloat16).astype(np.float32)
    l = (r - m).astype(ml_dtypes.bfloat16).astype(np.float32)
    return h, m, l


def _prep_inputs(gt, pred):
    import ml_dtypes
    gt = np.asarray(gt, dtype=np.float32)
    pred = np.asarray(pred, dtype=np.float32)
    xx = np.sum(gt * gt, axis=-1)      # (B, N)
    yy = np.sum(pred * pred, axis=-1)  # (B, N)
    onesN = np.ones((N,), dtype=np.float32)

    in_maps = []
    for c in range(NCORES):
        gtp = np.empty((BL, KR, N), dtype=np.float32)
        prd = np.empty((BL, KR, N), dtype=np.float32)
        for b in range(BL):
            g = 2 * c + b
            k = 0
            for cc in range(3):
                g0, g1, g2 = _split3(-2.0 * gt[g, :, cc])
                p0, p1, p2 = _split3(pred[g, :, cc])
                # 6 dominant cross products of (g0+g1+g2)*(p0+p1+p2)
                for gl, pl in ((g0, p0), (g0, p1), (g1, p0),
                               (g0, p2), (g1, p1), (g2, p0)):
                    gtp[b, k] = gl
                    prd[b, k] = pl
                    k += 1
            x0, x1, x2 = _split3(xx[g])
            for xl in (x0, x1, x2):
                gtp[b, k] = xl
                prd[b, k] = onesN
                k += 1
            y0, y1, y2 = _split3(yy[g])
            for yl in (y0, y1, y2):
                gtp[b, k] = onesN
                prd[b, k] = yl
                k += 1
            assert k == KR
        in_maps.append({"gtp": gtp.astype(ml_dtypes.bfloat16),
                        "prd": prd.astype(ml_dtypes.bfloat16)})
    return in_maps


def kernel(gt, pred, trace=False):
    from concourse.bass_utils import run_bass_kernel_spmd

    if "nc" not in _CACHE:
        _CACHE["nc"] = _build_nc()
    nc = _CACHE["nc"]
    in_maps = _prep_inputs(gt, pred)
    r = run_bass_kernel_spmd(nc, in_maps, list(range(NCORES)), trace=trace)
    total = sum(float(m["out"][0, 0]) for m in r.results)
    loss = np.float32(total / (B * N))
    if trace:
        return loss, r
    return loss


# revision 17
# speedup vs baseline: 1.0291x; 1.0046x over previous
"""Chamfer (GCC) loss kernel for Trainium2, 8-core data-parallel.

Problem: gt, pred: (16, 4096, 3) fp32. loss = mean(min_m d2[b,n,m]) + mean(min_n d2[b,n,m])
with d2 the squared pairwise distance matrix per batch.

Strategy per core (2 batches/core, data-parallel over batch):
  - d2 tiles are produced by the PE as a K=24 bf16 matmul. Each fp32 factor is
    split into 3 bf16 levels (hi/mid/lo) on the host; the 6 dominant cross
    products per coordinate plus 3-level xx and yy rows reproduce
    d2 = xx + yy - 2*gt.pred to ~1e-5 abs while streaming at full PE rate
    (bf16 matmul cost is output-bound, not K-bound).
  - ACT (scalar engine) drains PSUM -> SBUF fp16 (the only engine free to do it).
  - DVE row-min: TT-min tree (2x_1p fp16) per [128,2048] chunk + one
    reduce_min, written per n-block into rowminsA/B[128, 32].
  - DVE col-min: running tensor_tensor(min) into colacc[128, 4096] fp16.
  - Col tail: PE transposes colacc 128x128 blocks into PSUM fp16, one
    reduce_min gives colmins[128, 32].
  - clamp max(d2,0) commutes with min -> applied after, via Relu on ACT with
    fused sum accumulation; partition sums via ones-vector matmul.
    Per-core scalar partial -> host sum / (B*N).
"""

import numpy as np

B = 16
N = 4096
NCORES = 8
BL = B // NCORES          # batches per core
NB = N // 128             # 32 n-blocks
CH = 2048                 # m-chunk width (4 psum banks)
NH = N // CH              # 2 chunks per n-block row
MMN = 512                 # matmul moving free dim (1 psum bank)
KR = 24                   # matmul contraction rows

_CACHE = {}


def _build_nc(n_batches=BL, n_nblocks=NB, do_tail=True):
    from contextlib import ExitStack
    import concourse.bacc as bacc
    import concourse.tile as tile
    from concourse import mybir, masks

    f32 = mybir.dt.float32
    bf16 = mybir.dt.bfloat16
    f16 = mybir.dt.float16
    MIN_OP = mybir.AluOpType.min

    nc = bacc.Bacc("TRN2", target_bir_lowering=False, debug=False)
    gtp_d = nc.dram_tensor("gtp", [BL, KR, N], bf16, kind="ExternalInput")
    prd_d = nc.dram_tensor("prd", [BL, KR, N], bf16, kind="ExternalInput")
    out_d = nc.dram_tensor("out", [1, 1], f32, kind="ExternalOutput")

    with tile.TileContext(nc) as tc, ExitStack() as ctx:
        io = ctx.enter_context(tc.tile_pool(name="io", bufs=1))
        chunks = ctx.enter_context(tc.tile_pool(name="chunks", bufs=4))
        tree = ctx.enter_context(tc.tile_pool(name="tree", bufs=3))
        accs = ctx.enter_context(tc.tile_pool(name="accs", bufs=2))
        psum = ctx.enter_context(tc.tile_pool(name="psum", bufs=2, space="PSUM"))
        psumt = ctx.enter_context(tc.tile_pool(name="psumt", bufs=1, space="PSUM"))

        # constants
        ident = io.tile([128, 128], f16, tag="ident")
        masks.make_identity(nc, ident[:])
        ones = io.tile([128, 1], f32, tag="ones")
        nc.gpsimd.memset(ones[:], 1.0)
        scat = io.tile([1, 2 * BL], f32, tag="scat")

        # inputs
        gts = []
        prds = []
        for b in range(BL):
            gt_sb = io.tile([KR, N], bf16, tag=f"gt{b}")
            prd_sb = io.tile([KR, N], bf16, tag=f"prd{b}")
            if b == 0:
                # first-needed slices land first so the PE can start early
                nc.sync.dma_start(gt_sb[:, 0:128], gtp_d[0][:, 0:128])
                nc.scalar.dma_start(prd_sb[:, 0:CH], prd_d[0][:, 0:CH])
                nc.sync.dma_start(prd_sb[:, CH:N], prd_d[0][:, CH:N])
                nc.scalar.dma_start(gt_sb[:, 128:N], gtp_d[0][:, 128:N])
            else:
                nc.gpsimd.dma_start(gt_sb[:], gtp_d[b])
                nc.gpsimd.dma_start(prd_sb[:], prd_d[b])
            gts.append(gt_sb)
            prds.append(prd_sb)

        def emit_tail(b, rowminsA, colacc):
            # dist1: sum over n of relu(rowmins)
            relu1 = accs.tile([128, NB], f32, tag="relu1")
            s1 = accs.tile([128, 1], f32, tag="s1")
            nc.scalar.activation(relu1[:], rowminsA[:],
                                 mybir.ActivationFunctionType.Relu,
                                 accum_out=s1[:])
            ps11 = psumt.tile([1, 1], f32, tag="pst")
            nc.tensor.matmul(ps11[:], ones[:], s1[:], start=True, stop=True)
            nc.vector.tensor_copy(scat[:, 2 * b:2 * b + 1], ps11[:])

            # dist2: transpose colacc, min over partitions, relu-sum.
            # pst lives in its own 2-bank pool so the tail never displaces the
            # main pipeline's psum slots.
            cmins = accs.tile([128, NB], f32, tag="cmins")
            for hh in range(2):
                pst = psumt.tile([128, N // 2], f16, tag="pst")
                for blk in range(NB // 2):
                    g = hh * (NB // 2) + blk
                    nc.tensor.transpose(pst[:, blk * 128:(blk + 1) * 128],
                                        colacc[:, g * 128:(g + 1) * 128],
                                        ident[:])
                nc.vector.tensor_reduce(
                    cmins[:, hh * (NB // 2):(hh + 1) * (NB // 2)],
                    pst[:].rearrange("p (b f) -> p b f", f=128),
                    axis=mybir.AxisListType.X, op=MIN_OP)
            relu2 = accs.tile([128, NB], f32, tag="relu2")
            s2 = accs.tile([128, 1], f32, tag="s2")
            nc.scalar.activation(relu2[:], cmins[:],
                                 mybir.ActivationFunctionType.Relu,
                                 accum_out=s2[:])
            ps12 = psumt.tile([1, 1], f32, tag="pst")
            nc.tensor.matmul(ps12[:], ones[:], s2[:], start=True, stop=True)
            nc.vector.tensor_copy(scat[:, 2 * b + 1:2 * b + 2], ps12[:])

        pending_tail = None
        for b in range(n_batches):
            gt_sb, prd_sb = gts[b], prds[b]
            rowminsA = accs.tile([128, NB], f32, tag="rowminsA")
            colacc = accs.tile([128, N], f16, tag="colacc")
            for nb in range(n_nblocks):
                if nb == 3 and pending_tail is not None:
                    emit_tail(*pending_tail)
                    pending_tail = None
                lhsT = gt_sb[:, nb * 128:(nb + 1) * 128]
                sb = chunks.tile([128, N], f16, tag="sb")
                m0 = 0
                for ch in (1536, 1536, 1024):   # 3+3+2 psum banks; 2 left for pst
                    ps = psum.tile([128, ch], f32, tag="ps")
                    for j in range(ch // MMN):
                        nc.tensor.matmul(
                            ps[:, j * MMN:(j + 1) * MMN],
                            lhsT,
                            prd_sb[:, m0 + j * MMN:m0 + (j + 1) * MMN],
                            start=True, stop=True,
                        )
                    nc.scalar.copy(sb[:, m0:m0 + ch], ps[:])
                    m0 += ch
                # col-direction: running elementwise min over n-blocks
                if nb == 0:
                    nc.vector.tensor_copy(colacc[:], sb[:])
                else:
                    nc.vector.tensor_tensor(colacc[:], colacc[:], sb[:], op=MIN_OP)
                # row-direction: TT-min tree over all 4096 m
                r0 = tree.tile([128, N // 2], f16, tag="r0")
                nc.vector.tensor_tensor(r0[:], sb[:, :N // 2], sb[:, N // 2:], op=MIN_OP)
                r1 = tree.tile([128, N // 4], f16, tag="r1")
                nc.vector.tensor_tensor(r1[:], r0[:, :N // 4], r0[:, N // 4:], op=MIN_OP)
                r2 = tree.tile([128, N // 8], f16, tag="r2")
                nc.vector.tensor_tensor(r2[:], r1[:, :N // 8], r1[:, N // 8:], op=MIN_OP)
                r3 = tree.tile([128, N // 16], f16, tag="r3")
                nc.vector.tensor_tensor(r3[:], r2[:, :N // 16], r2[:, N // 16:], op=MIN_OP)
                nc.vector.tensor_reduce(rowminsA[:, nb:nb + 1], r3[:],
                                        axis=mybir.AxisListType.X, op=MIN_OP)

            if not do_tail:
                nc.vector.tensor_copy(scat[:, 2 * b:2 * b + 1], rowminsA[:1, :1])
                nc.vector.tensor_copy(scat[:, 2 * b + 1:2 * b + 2], colacc[:1, :1])
                continue
            pending_tail = (b, rowminsA, colacc)

        if pending_tail is not None:
            emit_tail(*pending_tail)
            pending_tail = None
        res = io.tile([1, 1], f32, tag="res")
        nc.vector.tensor_reduce(res[:], scat[:],
                                axis=mybir.AxisListType.X,
                                op=mybir.AluOpType.add)
        nc.sync.dma_start(out_d[:], res[:])

    nc.compile()
    return nc


def _split3(x):
    """Split fp32 array into 3 bf16 levels (returned as fp32 arrays)."""
    import ml_dtypes
    h = x.astype(ml_dtypes.bfloat16).astype(np.float32)
    r = x - h
    m = r.astype(ml_dtypes.bfloat16).astype(np.float32)
    l = (r - m).astype(ml_dtypes.bfloat16).astype(np.float32)
    return h, m, l


def _prep_inputs(gt, pred):
    import ml_dtypes
    gt = np.asarray(gt, dtype=np.float32)
    pred = np.asarray(pred, dtype=np.float32)
    xx = np.sum(gt * gt, axis=-1)      # (B, N)
    yy = np.sum(pred * pred, axis=-1)  # (B, N)
    onesN = np.ones((N,), dtype=np.float32)

    in_maps = []
    for c in range(NCORES):
        gtp = np.empty((BL, KR, N), dtype=np.float32)
        prd = np.empty((BL, KR, N), dtype=np.float32)
        for b in range(BL):
            g = 2 * c + b
            k = 0
            for cc in range(3):
                g0, g1, g2 = _split3(-2.0 * gt[g, :, cc])
                p0, p1, p2 = _split3(pred[g, :, cc])
                # 6 dominant cross products of (g0+g1+g2)*(p0+p1+p2)
                for gl, pl in ((g0, p0), (g0, p1), (g1, p0),
                               (g0, p2), (g1, p1), (g2, p0)):
                    gtp[b, k] = gl
                    prd[b, k] = pl
                    k += 1
            x0, x1, x2 = _split3(xx[g])
            for xl in (x0, x1, x2):
                gtp[b, k] = xl
                prd[b, k] = onesN
                k += 1
            y0, y1, y2 = _split3(yy[g])
            for yl in (y0, y1, y2):
                gtp[b, k] = onesN
                prd[b, k] = yl
                k += 1
            assert k == KR
        in_maps.append({"gtp": gtp.astype(ml_dtypes.bfloat16),
                        "prd": prd.astype(ml_dtypes.bfloat16)})
    return in_maps


def kernel(gt, pred, trace=False):
    from concourse.bass_utils import run_bass_kernel_spmd

    if "nc" not in _CACHE:
        _CACHE["nc"] = _build_nc()
    nc = _CACHE["nc"]
    in_maps = _prep_inputs(gt, pred)
    r = run_bass_kernel_spmd(nc, in_maps, list(range(NCORES)), trace=trace)
    total = sum(float(m["out"][0, 0]) for m in r.results)
    loss = np.float32(total / (B * N))
    if trace:
        return loss, r
    return loss


# revision 18
# speedup vs baseline: 1.0380x; 1.0087x over previous
"""Chamfer (GCC) loss kernel for Trainium2, 8-core data-parallel.

Problem: gt, pred: (16, 4096, 3) fp32. loss = mean(min_m d2[b,n,m]) + mean(min_n d2[b,n,m])
with d2 the squared pairwise distance matrix per batch.

Strategy per core (2 batches/core, data-parallel over batch):
  - d2 tiles are produced by the PE as a K=24 bf16 matmul. Each fp32 factor is
    split into 3 bf16 levels (hi/mid/lo) on the host; the 6 dominant cross
    products per coordinate plus 3-level xx and yy rows reproduce
    d2 = xx + yy - 2*gt.pred to ~1e-5 abs while streaming at full PE rate
    (bf16 matmul cost is output-bound, not K-bound).
  - ACT (scalar engine) drains PSUM -> SBUF fp16 (the only engine free to do it).
  - DVE row-min: TT-min tree (2x_1p fp16) per [128,2048] chunk + one
    reduce_min, written per n-block into rowminsA/B[128, 32].
  - DVE col-min: running tensor_tensor(min) into colacc[128, 4096] fp16.
  - Col tail: PE transposes colacc 128x128 blocks into PSUM fp16, one
    reduce_min gives colmins[128, 32].
  - clamp max(d2,0) commutes with min -> applied after, via Relu on ACT with
    fused sum accumulation; partition sums via ones-vector matmul.
    Per-core scalar partial -> host sum / (B*N).
"""

import numpy as np

B = 16
N = 4096
NCORES = 8
BL = B // NCORES          # batches per core
NB = N // 128             # 32 n-blocks
CH = 2048                 # m-chunk width (4 psum banks)
NH = N // CH              # 2 chunks per n-block row
MMN = 512                 # matmul moving free dim (1 psum bank)
KR = 24                   # matmul contraction rows

_CACHE = {}


def _build_nc(n_batches=BL, n_nblocks=NB, do_tail=True):
    from contextlib import ExitStack
    import concourse.bacc as bacc
    import concourse.tile as tile
    from concourse import mybir, masks

    f32 = mybir.dt.float32
    bf16 = mybir.dt.bfloat16
    f16 = mybir.dt.float16
    MIN_OP = mybir.AluOpType.min

    nc = bacc.Bacc("TRN2", target_bir_lowering=False, debug=False)
    gtp_d = nc.dram_tensor("gtp", [BL, KR, N], bf16, kind="ExternalInput")
    prd_d = nc.dram_tensor("prd", [BL, KR, N], bf16, kind="ExternalInput")
    out_d = nc.dram_tensor("out", [1, 1], f32, kind="ExternalOutput")

    with tile.TileContext(nc) as tc, ExitStack() as ctx:
        io = ctx.enter_context(tc.tile_pool(name="io", bufs=1))
        chunks = ctx.enter_context(tc.tile_pool(name="chunks", bufs=4))
        tree = ctx.enter_context(tc.tile_pool(name="tree", bufs=3))
        accs = ctx.enter_context(tc.tile_pool(name="accs", bufs=2))
        psum = ctx.enter_context(tc.tile_pool(name="psum", bufs=2, space="PSUM"))
        psumt = ctx.enter_context(tc.tile_pool(name="psumt", bufs=2, space="PSUM"))

        # constants
        ident = io.tile([128, 128], f16, tag="ident")
        masks.make_identity(nc, ident[:])
        ones = io.tile([128, 1], f32, tag="ones")
        nc.gpsimd.memset(ones[:], 1.0)
        scat = io.tile([1, 2 * BL], f32, tag="scat")

        # inputs
        gts = []
        prds = []
        for b in range(BL):
            gt_sb = io.tile([KR, N], bf16, tag=f"gt{b}")
            prd_sb = io.tile([KR, N], bf16, tag=f"prd{b}")
            if b == 0:
                # first-needed slices land first so the PE can start early
                nc.sync.dma_start(gt_sb[:, 0:128], gtp_d[0][:, 0:128])
                nc.scalar.dma_start(prd_sb[:, 0:CH], prd_d[0][:, 0:CH])
                nc.sync.dma_start(prd_sb[:, CH:N], prd_d[0][:, CH:N])
                nc.scalar.dma_start(gt_sb[:, 128:N], gtp_d[0][:, 128:N])
            else:
                nc.gpsimd.dma_start(gt_sb[:], gtp_d[b])
                nc.gpsimd.dma_start(prd_sb[:], prd_d[b])
            gts.append(gt_sb)
            prds.append(prd_sb)

        def emit_tail(b, rowminsA, colacc):
            # dist1: sum over n of relu(rowmins)
            relu1 = accs.tile([128, NB], f32, tag="relu1")
            s1 = accs.tile([128, 1], f32, tag="s1")
            nc.scalar.activation(relu1[:], rowminsA[:],
                                 mybir.ActivationFunctionType.Relu,
                                 accum_out=s1[:])
            ps11 = psumt.tile([1, 1], f32, tag="pst")
            nc.tensor.matmul(ps11[:], ones[:], s1[:], start=True, stop=True)
            nc.vector.tensor_copy(scat[:, 2 * b:2 * b + 1], ps11[:])

            # dist2: transpose colacc, min over partitions, relu-sum.
            # pst lives in its own 2-bank pool so the tail never displaces the
            # main pipeline's psum slots.
            cmins = accs.tile([128, NB], f32, tag="cmins")
            QB = NB // 4
            for qq in range(4):
                pst = psumt.tile([128, QB * 128], f16, tag="pst")
                for blk in range(QB):
                    g = qq * QB + blk
                    nc.tensor.transpose(pst[:, blk * 128:(blk + 1) * 128],
                                        colacc[:, g * 128:(g + 1) * 128],
                                        ident[:])
                nc.vector.tensor_reduce(
                    cmins[:, qq * QB:(qq + 1) * QB],
                    pst[:].rearrange("p (b f) -> p b f", f=128),
                    axis=mybir.AxisListType.X, op=MIN_OP)
            relu2 = accs.tile([128, NB], f32, tag="relu2")
            s2 = accs.tile([128, 1], f32, tag="s2")
            nc.scalar.activation(relu2[:], cmins[:],
                                 mybir.ActivationFunctionType.Relu,
                                 accum_out=s2[:])
            ps12 = psumt.tile([1, 1], f32, tag="pst")
            nc.tensor.matmul(ps12[:], ones[:], s2[:], start=True, stop=True)
            nc.vector.tensor_copy(scat[:, 2 * b + 1:2 * b + 2], ps12[:])

        pending_tail = None
        for b in range(n_batches):
            gt_sb, prd_sb = gts[b], prds[b]
            rowminsA = accs.tile([128, NB], f32, tag="rowminsA")
            colacc = accs.tile([128, N], f16, tag="colacc")
            for nb in range(n_nblocks):
                if nb == 3 and pending_tail is not None:
                    emit_tail(*pending_tail)
                    pending_tail = None
                lhsT = gt_sb[:, nb * 128:(nb + 1) * 128]
                sb = chunks.tile([128, N], f16, tag="sb")
                m0 = 0
                for ch in (1536, 1536, 1024):   # 3+3+2 psum banks; 2 left for pst
                    ps = psum.tile([128, ch], f32, tag="ps")
                    for j in range(ch // MMN):
                        nc.tensor.matmul(
                            ps[:, j * MMN:(j + 1) * MMN],
                            lhsT,
                            prd_sb[:, m0 + j * MMN:m0 + (j + 1) * MMN],
                            start=True, stop=True,
                        )
                    nc.scalar.copy(sb[:, m0:m0 + ch], ps[:])
                    m0 += ch
                # col-direction: running elementwise min over n-blocks
                if nb == 0:
                    nc.vector.tensor_copy(colacc[:], sb[:])
                else:
                    nc.vector.tensor_tensor(colacc[:], colacc[:], sb[:], op=MIN_OP)
                # row-direction: TT-min tree over all 4096 m
                r0 = tree.tile([128, N // 2], f16, tag="r0")
                nc.vector.tensor_tensor(r0[:], sb[:, :N // 2], sb[:, N // 2:], op=MIN_OP)
                r1 = tree.tile([128, N // 4], f16, tag="r1")
                nc.vector.tensor_tensor(r1[:], r0[:, :N // 4], r0[:, N // 4:], op=MIN_OP)
                r2 = tree.tile([128, N // 8], f16, tag="r2")
                nc.vector.tensor_tensor(r2[:], r1[:, :N // 8], r1[:, N // 8:], op=MIN_OP)
                r3 = tree.tile([128, N // 16], f16, tag="r3")
                nc.vector.tensor_tensor(r3[:], r2[:, :N // 16], r2[:, N // 16:], op=MIN_OP)
                nc.vector.tensor_reduce(rowminsA[:, nb:nb + 1], r3[:],
                                        axis=mybir.AxisListType.X, op=MIN_OP)

            if not do_tail:
                nc.vector.tensor_copy(scat[:, 2 * b:2 * b + 1], rowminsA[:1, :1])
                nc.vector.tensor_copy(scat[:, 2 * b + 1:2 * b + 2], colacc[:1, :1])
                continue
            pending_tail = (b, rowminsA, colacc)

        if pending_tail is not None:
            emit_tail(*pending_tail)
            pending_tail = None
        res = io.tile([1, 1], f32, tag="res")
        nc.vector.tensor_reduce(res[:], scat[:],
                                axis=mybir.AxisListType.X,
                                op=mybir.AluOpType.add)
        nc.sync.dma_start(out_d[:], res[:])

    nc.compile()
    return nc


def _split3(x):
    """Split fp32 array into 3 bf16 levels (returned as fp32 arrays)."""
    import ml_dtypes
    h = x.astype(ml_dtypes.bfloat16).astype(np.float32)
    r = x - h
    m = r.astype(ml_dtypes.bfloat16).astype(np.float32)
    l = (r - m).astype(ml_dtypes.bfloat16).astype(np.float32)
    return h, m, l


def _prep_inputs(gt, pred):
    import ml_dtypes
    gt = np.asarray(gt, dtype=np.float32)
    pred = np.asarray(pred, dtype=np.float32)
    xx = np.sum(gt * gt, axis=-1)      # (B, N)
    yy = np.sum(pred * pred, axis=-1)  # (B, N)
    onesN = np.ones((N,), dtype=np.float32)

    in_maps = []
    for c in range(NCORES):
        gtp = np.empty((BL, KR, N), dtype=np.float32)
        prd = np.empty((BL, KR, N), dtype=np.float32)
        for b in range(BL):
            g = 2 * c + b
            k = 0
            for cc in range(3):
                g0, g1, g2 = _split3(-2.0 * gt[g, :, cc])
                p0, p1, p2 = _split3(pred[g, :, cc])
                # 6 dominant cross products of (g0+g1+g2)*(p0+p1+p2)
                for gl, pl in ((g0, p0), (g0, p1), (g1, p0),
                               (g0, p2), (g1, p1), (g2, p0)):
                    gtp[b, k] = gl
                    prd[b, k] = pl
                    k += 1
            x0, x1, x2 = _split3(xx[g])
            for xl in (x0, x1, x2):
                gtp[b, k] = xl
                prd[b, k] = onesN
                k += 1
            y0, y1, y2 = _split3(yy[g])
            for yl in (y0, y1, y2):
                gtp[b, k] = onesN
                prd[b, k] = yl
                k += 1
            assert k == KR
        in_maps.append({"gtp": gtp.astype(ml_dtypes.bfloat16),
                        "prd": prd.astype(ml_dtypes.bfloat16)})
    return in_maps


def kernel(gt, pred, trace=False):
    from concourse.bass_utils import run_bass_kernel_spmd

    if "nc" not in _CACHE:
        _CACHE["nc"] = _build_nc()
    nc = _CACHE["nc"]
    in_maps = _prep_inputs(gt, pred)
    r = run_bass_kernel_spmd(nc, in_maps, list(range(NCORES)), trace=trace)
    total = sum(float(m["out"][0, 0]) for m in r.results)
    loss = np.float32(total / (B * N))
    if trace:
        return loss, r
    return loss
